# revision 7
# baseline (speedup 1.0000x reference)
"""Multi-head causal attention with RoPE on 8 Trainium2 NeuronCores.

Sharding: 2 (batch) x 4 (head-groups of 4 heads). Each core computes
QKV projections, RoPE, flash-style causal attention and its slice of the
output projection for one batch and 4 heads; partial outputs are summed
on the host (row-sharded out_proj => partial-sum reduction).

Device layout choices (everything host-prepped to avoid on-device
transposes, fp32 has no DMA-transpose path):
  - x is passed pre-transposed per batch: xT [D, S] bf16
  - Q^T, K^T computed as [head_dim, S] (lhsT = W tile, rhs = xT)
  - V computed natural [S, head_dim] (lhsT = xT tile, rhs = Wv)
  - scores computed transposed [k, q]; softmax sum over k (partitions)
    via ones-vector matmul; normalization broadcast via rank-1 matmul
  - RoPE rotate-half done with a signed 128x128 permutation matmul
"""

import math
import sys

import numpy as np

try:
    import concourse.bass as bass  # noqa: F401
except Exception:
    sys.path.insert(0, "/opt/trn_rl_repo")

import ml_dtypes

P = 128
B = 2
S = 2048
D = 2048
H = 16
HEAD = 128
N_CORES = 8
HG = 4            # head groups (tensor-parallel dimension)
HPG = H // HG     # heads per group = 4
DG = HPG * HEAD   # group width = 512
SG = 512          # q-group (free dim) size
DOUT = 2048

BF16 = ml_dtypes.bfloat16


def _emit(tc, io, cfg):
    """Emit the per-core program. io: dict of dram APs. cfg: sizes."""
    import concourse.mybir as mybir

    nc = tc.nc
    bf = mybir.dt.bfloat16
    f32 = mybir.dt.float32
    Exp = mybir.ActivationFunctionType.Exp

    s = cfg["S"]
    d = cfg["D"]
    dout = cfg["DOUT"]
    di_t = d // P          # d_in k-tiles
    st = s // P            # seq 128-tiles
    nsg = s // SG          # seq 512-groups
    nos = dout // SG       # out column slices
    inv_sqrt_hd = 1.0 / math.sqrt(HEAD)

    xT = io["xT"].rearrange("(o p) s -> p o s", p=P)
    wq = io["wq"].rearrange("(o p) n -> p o n", p=P)
    wk = io["wk"].rearrange("(o p) n -> p o n", p=P)
    wv = io["wv"].rearrange("(o p) n -> p o n", p=P)
    wo = io["wo"].rearrange("(o p) n -> p o n", p=P)

    const = tc.alloc_tile_pool(name="const", bufs=1)
    stores = tc.alloc_tile_pool(name="stores", bufs=1)
    ps_main = tc.alloc_tile_pool(name="ps_main", bufs=6, space="PSUM")
    ps_sum = tc.alloc_tile_pool(name="ps_sum", bufs=2, space="PSUM")

    # ---- constants ----
    cos_sb = const.tile([P, s], bf, tag="cos")
    nc.sync.dma_start(cos_sb[:], io["cosT"][:])
    sin_sb = const.tile([P, s], bf, tag="sin")
    nc.sync.dma_start(sin_sb[:], io["sinT"][:])
    rot_sb = const.tile([P, P], bf, tag="rot")
    nc.sync.dma_start(rot_sb[:], io["rot"][:])
    mask_sb = const.tile([P, HG, SG], bf, tag="mask")
    nc.sync.dma_start(mask_sb[:], io["masks"][:])
    ones_bf_sb = const.tile([P, 1], bf, tag="ones_bf")
    nc.sync.dma_start(ones_bf_sb[:], io["ones_bf"][:])
    ones_f_sb = const.tile([1, P], f32, tag="ones_f")
    nc.sync.dma_start(ones_f_sb[:], io["ones_f"][:])

    wv_sb = const.tile([P, di_t, DG], bf, tag="wv")
    for o in range(di_t):
        nc.sync.dma_start(wv_sb[:, o, :], wv[:, o, :])

    # persistent activation stores
    qt_sb = stores.tile([P, HPG, s], bf, tag="qt")
    kt_sb = stores.tile([P, HPG, s], bf, tag="kt")
    v_sb = stores.tile([P, st, DG], bf, tag="v")
    ctx_sb = stores.tile([P, HPG, s], bf, tag="ctx")

    # ---- phase 1: projections + RoPE ----
    with tc.tile_pool(name="xt", bufs=1) as xtp, \
         tc.tile_pool(name="wqk", bufs=2) as wqkp, \
         tc.tile_pool(name="p1tmp", bufs=4) as p1tmp:
        xt_sb = xtp.tile([P, di_t, s], bf, tag="xt")
        for o in range(di_t):
            nc.sync.dma_start(xt_sb[:, o, :], xT[:, o, :])

        # V natural layout: [s_tile, DG]
        for si in range(st):
            pv = ps_main.tile([P, SG], f32, tag="ps")
            for o in range(di_t):
                nc.tensor.matmul(
                    pv[:, :DG],
                    lhsT=xt_sb[:, o, si * P:(si + 1) * P],
                    rhs=wv_sb[:, o, :],
                    start=(o == 0),
                    stop=(o == di_t - 1),
                )
            nc.vector.tensor_copy(v_sb[:, si, :], pv[:, :DG])

        # Q^T, K^T with RoPE, per head
        for h in range(HPG):
            wq_t = wqkp.tile([P, di_t, P], bf, tag="wq")
            wk_t = wqkp.tile([P, di_t, P], bf, tag="wk")
            for o in range(di_t):
                nc.sync.dma_start(wq_t[:, o, :], wq[:, o, h * P:(h + 1) * P])
                nc.sync.dma_start(wk_t[:, o, :], wk[:, o, h * P:(h + 1) * P])
            for g in range(nsg):
                sl = slice(g * SG, (g + 1) * SG)
                for which, w_t, dst in (("q", wq_t, qt_sb), ("k", wk_t, kt_sb)):
                    pq = ps_main.tile([P, SG], f32, tag="ps")
                    for o in range(di_t):
                        nc.tensor.matmul(
                            pq,
                            lhsT=w_t[:, o, :],
                            rhs=xt_sb[:, o, sl],
                            start=(o == 0),
                            stop=(o == di_t - 1),
                        )
                    qa = p1tmp.tile([P, SG], bf, tag="qa")
                    nc.scalar.copy(qa, pq)
                    pr = ps_main.tile([P, SG], f32, tag="ps")
                    nc.tensor.matmul(pr, lhsT=rot_sb, rhs=qa, start=True, stop=True)
                    t1 = p1tmp.tile([P, SG], bf, tag="t1")
                    nc.vector.tensor_mul(t1, qa, cos_sb[:, sl])
                    t2 = p1tmp.tile([P, SG], bf, tag="t2")
                    nc.vector.tensor_mul(t2, pr, sin_sb[:, sl])
                    nc.vector.tensor_add(dst[:, h, sl], t1, t2)

    # ---- phase 2: attention (scores transposed: [k, q]) ----
    with tc.tile_pool(name="p2tmp", bufs=4) as p2tmp, \
         tc.tile_pool(name="p2rb", bufs=2) as p2rb:
        for h in range(HPG):
            for g in range(nsg):
                qsl = slice(g * SG, (g + 1) * SG)
                jmax = min((g + 1) * SG // P, st)
                pctx = ps_main.tile([P, SG], f32, tag="ps")
                psum_l = ps_sum.tile([1, SG], f32, tag="l")
                for j in range(jmax):
                    pscore = ps_main.tile([P, SG], f32, tag="ps")
                    nc.tensor.matmul(
                        pscore,
                        lhsT=kt_sb[:, h, j * P:(j + 1) * P],
                        rhs=qt_sb[:, h, qsl],
                        start=True,
                        stop=True,
                    )
                    at = p2tmp.tile([P, SG], bf, tag="at")
                    nc.scalar.activation(at, pscore, Exp, scale=inv_sqrt_hd)
                    r = j - (g * SG // P)
                    if r >= 0:
                        nc.vector.tensor_mul(at, at, mask_sb[:, r, :])
                    nc.tensor.matmul(
                        pctx,
                        lhsT=v_sb[:, j, h * P:(h + 1) * P],
                        rhs=at,
                        start=(j == 0),
                        stop=(j == jmax - 1),
                    )
                    nc.tensor.matmul(
                        psum_l,
                        lhsT=ones_bf_sb[:, 0:1],
                        rhs=at,
                        start=(j == 0),
                        stop=(j == jmax - 1),
                    )
                rec = p2rb.tile([1, SG], f32, tag="rec")
                nc.vector.reciprocal(rec, psum_l)
                pb = ps_main.tile([P, SG], f32, tag="ps")
                nc.tensor.matmul(pb, lhsT=ones_f_sb, rhs=rec, start=True, stop=True)
                rb = p2rb.tile([P, SG], f32, tag="rb")
                nc.scalar.copy(rb, pb)
                nc.vector.tensor_mul(ctx_sb[:, h, qsl], pctx, rb)

    # ---- phase 3: output projection ----
    with tc.tile_pool(name="wo", bufs=1) as wop, \
         tc.tile_pool(name="outp", bufs=3) as outp:
        wo_sb = wop.tile([P, HPG, dout], bf, tag="wo")
        for o in range(HPG):
            nc.sync.dma_start(wo_sb[:, o, :], wo[:, o, :])
        for qt in range(st):
            for dsl in range(nos):
                po = ps_main.tile([P, SG], f32, tag="ps")
                for h in range(HPG):
                    nc.tensor.matmul(
                        po,
                        lhsT=ctx_sb[:, h, qt * P:(qt + 1) * P],
                        rhs=wo_sb[:, h, dsl * SG:(dsl + 1) * SG],
                        start=(h == 0),
                        stop=(h == HPG - 1),
                    )
                ob = outp.tile([P, SG], f32, tag="ob")
                nc.scalar.copy(ob, po)
                nc.sync.dma_start(
                    io["out"][qt * P:(qt + 1) * P, dsl * SG:(dsl + 1) * SG], ob
                )

    for pool in (ps_sum, ps_main, stores, const):
        pool.release()


def build_program(cfg=None):
    import concourse.bacc as bacc
    import concourse.mybir as mybir
    import concourse.tile as tile

    cfg = cfg or {"S": S, "D": D, "DOUT": DOUT}
    bf = mybir.dt.bfloat16
    f32 = mybir.dt.float32
    nc = bacc.Bacc()
    io = {
        "xT": nc.dram_tensor("xT", [cfg["D"], cfg["S"]], bf, kind="ExternalInput"),
        "wq": nc.dram_tensor("wq", [cfg["D"], DG], bf, kind="ExternalInput"),
        "wk": nc.dram_tensor("wk", [cfg["D"], DG], bf, kind="ExternalInput"),
        "wv": nc.dram_tensor("wv", [cfg["D"], DG], bf, kind="ExternalInput"),
        "wo": nc.dram_tensor("wo", [DG, cfg["DOUT"]], bf, kind="ExternalInput"),
        "cosT": nc.dram_tensor("cosT", [P, cfg["S"]], bf, kind="ExternalInput"),
        "sinT": nc.dram_tensor("sinT", [P, cfg["S"]], bf, kind="ExternalInput"),
        "rot": nc.dram_tensor("rot", [P, P], bf, kind="ExternalInput"),
        "masks": nc.dram_tensor("masks", [P, HG, SG], bf, kind="ExternalInput"),
        "ones_bf": nc.dram_tensor("ones_bf", [P, 1], bf, kind="ExternalInput"),
        "ones_f": nc.dram_tensor("ones_f", [1, P], f32, kind="ExternalInput"),
        "out": nc.dram_tensor(
            "out", [cfg["S"], cfg["DOUT"]], f32, kind="ExternalOutput"
        ),
    }
    with tile.TileContext(nc) as tc:
        _emit(tc, io, cfg)
    nc.finalize()
    return nc


def host_constants(s=S):
    inv = 1.0 / (10000.0 ** (np.arange(0, HEAD, 2, dtype=np.float32) / HEAD))
    pos = np.arange(s, dtype=np.float32)
    ang = pos[:, None] * inv[None, :]
    ang = np.concatenate([ang, ang], axis=-1)          # (s, HEAD)
    cosT = np.cos(ang).T.astype(BF16).copy()           # (HEAD, s)
    sinT = np.sin(ang).T.astype(BF16).copy()
    rot = np.zeros((HEAD, HEAD), np.float32)
    for dd in range(64):
        rot[dd, dd + 64] = -1.0
        rot[dd + 64, dd] = 1.0
    rotT = rot.T.astype(BF16).copy()                   # lhsT for out = rot @ q
    kk = np.arange(P)[:, None, None]
    rr = np.arange(HG)[None, :, None]
    qq = np.arange(SG)[None, None, :]
    masks = (kk <= qq - P * rr).astype(BF16)           # (P, HG, SG)
    ones_bf = np.ones((P, 1), BF16)
    ones_f = np.ones((1, P), np.float32)
    return cosT, sinT, rotT, masks, ones_bf, ones_f


def kernel(x, W_query, W_key, W_value, W_out):
    from concourse.bass_utils import run_bass_kernel_spmd

    x = np.asarray(x)
    in_dtype = x.dtype
    nc = build_program()
    cosT, sinT, rotT, masks, ones_bf, ones_f = host_constants()

    xTb = [np.ascontiguousarray(np.asarray(x[b]).T).astype(BF16) for b in range(B)]
    in_maps = []
    for core in range(N_CORES):
        b, g = divmod(core, HG)
        gsl = slice(g * DG, (g + 1) * DG)
        in_maps.append({
            "xT": xTb[b],
            "wq": np.asarray(W_query)[:, gsl].astype(BF16).copy(),
            "wk": np.asarray(W_key)[:, gsl].astype(BF16).copy(),
            "wv": np.asarray(W_value)[:, gsl].astype(BF16).copy(),
            "wo": np.asarray(W_out)[gsl, :].astype(BF16).copy(),
            "cosT": cosT, "sinT": sinT, "rot": rotT, "masks": masks,
            "ones_bf": ones_bf, "ones_f": ones_f,
        })

    res = run_bass_kernel_spmd(nc, in_maps, core_ids=list(range(N_CORES)))
    out = np.zeros((B, S, DOUT), np.float32)
    for core in range(N_CORES):
        b = core // HG
        out[b] += res.results[core]["out"]
    return out.astype(in_dtype, copy=False)
